# revision 22
# baseline (speedup 1.0000x reference)
"""GATv2 molecular-graph kernel for 8 TRN2 NeuronCores (SPMD, data-parallel).

Host side (layout only — all reference FLOPs run on device): edges are
sorted by destination node and partitioned into 8 contiguous ranges with
~equal edge counts.  Per core, edges are packed into chunks of <=256 edge
slots covering <=128 distinct destination nodes; two chunks form a "pair"
(512 edge slots) which is the device work unit.  For each edge slot the
host gathers the RAW inputs (x[src], x[dst], edge_attr) into dense bf16
tensors, so the device needs no indirect DMA at all.

Device pipeline per pair (transposed feature-major layout):
  hsT/hdT = relu(wencA^T @ xsdT)            encoder (bn folded, bias via
                                            appended all-ones feature row)
  X^T_h   = wl2_h^T hs + wr2_h^T hd + we2_h^T ea    [C=128, 512 edges]
            (weights pre-scaled by 0.4*|att|, col-signs NOT permuted)
  A_h     = |X^T_h|                          (scalar/vector engines)
  dfq     = t1 (3 small matmuls) + sum_c sign(att_c)*A_h  (4 sigma-matmuls
            with single-column lhsT landing on PSUM partitions 0/32/64/96)
            = full GATv2 logits: 0.6*(z@att) + 0.4*sum|z_c att_c|
  ex      = exp(dfq)  (no segment-max shift; logits are O(0.1))
  gf      = [gfold*ex | ex]  where gfold = hs @ (Wl_h @ Wp_h)  [e,96]
  acc     = S^T @ gf  per subtile (S = slot-indicator built on gpsimd)
  out     = sum_h acc_h * 1/(denom_h)       (vector engine)
Softmax denominators ride the last 4 columns of gf.  Host adds the
constant terms (bias_conv@Wp + bp, and bl@Wp gated by deg>0).
"""

import numpy as np
import ml_dtypes

import concourse.bacc as bacc
import concourse.tile as tile
from concourse import mybir
import concourse.bass as bass
from concourse.bass_utils import run_bass_kernel_spmd
from concourse.masks import make_identity

P = 128
N_CORES = 8
EPC = 256            # edge slots per chunk
EPP = 512            # edge slots per pair (2 chunks)
SLOTS = 128          # max distinct dst nodes per chunk
NEG_SLOPE = 0.2
BN_EPS = 1e-5

FP = mybir.dt.float32
BF = mybir.dt.bfloat16
I32 = mybir.dt.int32
BF_NP = ml_dtypes.bfloat16
F8_NP = ml_dtypes.float8_e4m3fn
F8 = mybir.dt.float8e4
DR = mybir.MatmulPerfMode.DoubleRow


# ----------------------------------------------------------------------------
# host-side weight folding
# ----------------------------------------------------------------------------

def _fold_weights(W_enc, b_enc, bn_gamma, bn_beta, bn_mean, bn_var,
                  Wl, bl, Wr, br, We, att, bias_conv, Wp, bp):
    D = W_enc.shape[1]
    H, C = att.shape
    HC = H * C
    OUT = Wp.shape[1]
    s = bn_gamma / np.sqrt(bn_var + BN_EPS)
    W_enc_f = W_enc * s[None, :]
    b_enc_f = (b_enc - bn_mean) * s + bn_beta
    wencA = np.concatenate([W_enc_f, b_enc_f[None, :]], 0)      # [33, D]

    att_abs = np.abs(att)                                       # [H, C]
    bsum = bl + br                                              # [HC]

    # abs-path weights: X = 0.4 * |att| * z   (z = hs@Wl + hd@Wr + ea@We + b)
    wl2 = np.zeros((D, HC), np.float32)
    wr2 = np.zeros((D, HC), np.float32)
    we2 = np.zeros((17, HC), np.float32)
    for h in range(H):
        blk = slice(h * C, (h + 1) * C)
        wl2[:, blk] = 0.4 * Wl[:, blk] * att_abs[h][None, :]
        wr2[:, blk] = 0.4 * Wr[:, blk] * att_abs[h][None, :]
        we2[:16, blk] = 0.4 * We[:, blk] * att_abs[h][None, :]
        we2[16, blk] = 0.4 * bsum[blk] * att_abs[h]

    sgm = np.where(att > 0, 1.0, -1.0).astype(np.float32).T     # [C, H]

    # linear logit path: t1 = 0.6 * (z @ att_h), landing on partition 32h
    wla = np.zeros((D, 97), np.float32)
    wra = np.zeros((D, 97), np.float32)
    wea = np.zeros((17, 97), np.float32)
    for h in range(H):
        blk = slice(h * C, (h + 1) * C)
        wla[:, 32 * h] = 0.6 * (Wl[:, blk] @ att[h])
        wra[:, 32 * h] = 0.6 * (Wr[:, blk] @ att[h])
        wea[:16, 32 * h] = 0.6 * (We[:, blk] @ att[h])
        wea[16, 32 * h] = 0.6 * (bsum[blk] @ att[h])

    # folded node->out transform per head
    wfold = np.concatenate(
        [Wl[:, h * C:(h + 1) * C] @ Wp[h * C:(h + 1) * C] for h in range(H)], 1
    )                                                           # [D, 96]

    cbl = bl @ Wp                       # [OUT]: * 1{deg>0}
    cc = bias_conv @ Wp + bp            # [OUT]: always

    # fp8 DoubleRow packings (x64 so entries clear the e4m3 subnormal floor;
    # PSUM accumulates f32, the exp applies 1/64)
    S8 = 64.0
    wlr8 = np.zeros((D, 4 * 256), np.float32)
    we28 = np.zeros((9, 4 * 256), np.float32)
    for h in range(H):
        blk = slice(h * C, (h + 1) * C)
        wlr8[:, h * 256:h * 256 + 128] = S8 * wl2[:, blk]
        wlr8[:, h * 256 + 128:(h + 1) * 256] = S8 * wr2[:, blk]
        we28[:, h * 256:h * 256 + 128] = S8 * we2[0:9, blk]
        we28[0:8, h * 256 + 128:(h + 1) * 256] = S8 * we2[9:17, blk]
    # DoubleRow lhsT planes padded to 112 cols (stride must be 16B-aligned)
    wlra8 = np.zeros((D, 224), np.float32)
    wlra8[:, 0:97] = S8 * wla
    wlra8[:, 112:209] = S8 * wra
    wea8 = np.zeros((9, 224), np.float32)
    wea8[:, 0:97] = S8 * wea[0:9]
    wea8[0:8, 112:209] = S8 * wea[9:17]
    return dict(wencA=wencA, sgm=sgm, wfold=wfold,
                wlr8=wlr8, we28=we28, wlra8=wlra8, wea8=wea8,
                cbl=cbl, cc=cc, H=H, C=C, OUT=OUT, D=D)


# ----------------------------------------------------------------------------
# host-side edge packing
# ----------------------------------------------------------------------------

def _prepare(x, edge_attr, edge_index):
    N = x.shape[0]
    E = edge_index.shape[1]
    src = np.asarray(edge_index[0], dtype=np.int64)
    dst = np.asarray(edge_index[1], dtype=np.int64)

    order = np.argsort(dst, kind="stable")
    src_s = src[order]
    dst_s = dst[order]
    ea_s = np.asarray(edge_attr, dtype=np.float32)[order]

    deg = np.bincount(dst, minlength=N)
    cum = np.concatenate([[0], np.cumsum(deg)])

    bounds = [0]
    for c in range(1, N_CORES):
        bounds.append(int(np.searchsorted(cum, E * c // N_CORES)))
    bounds.append(N)

    xf = np.asarray(x, dtype=np.float32)

    cores = []
    for c in range(N_CORES):
        n0, n1 = bounds[c], bounds[c + 1]
        # chunking: walk deg>0 nodes; <=SLOTS nodes and <=EPC edges per chunk
        chunks = []          # list of (list-of-node-ids, e_start, e_end)
        cur_nodes = []
        ce0 = int(cum[n0])
        ecnt = 0
        for n in range(n0, n1):
            d = int(deg[n])
            if d == 0:
                continue
            if d > EPC:
                raise RuntimeError("node degree exceeds chunk capacity")
            if len(cur_nodes) >= SLOTS or ecnt + d > EPC:
                chunks.append((cur_nodes, ce0, ce0 + ecnt))
                ce0 += ecnt
                cur_nodes = []
                ecnt = 0
            cur_nodes.append(n)
            ecnt += d
        if cur_nodes:
            chunks.append((cur_nodes, ce0, ce0 + ecnt))
        cores.append(dict(chunks=chunks, n0=n0, n1=n1))

    NCH = max(len(cd["chunks"]) for cd in cores)
    NP = (NCH + 1) // 2

    in_maps = []
    for cd in cores:
        chunks = cd["chunks"]
        # layout per pair p: [src slots (512) | dst slots (512)]
        xsd = np.zeros((33, NP * 1024), np.float32)
        # ea in fp8 DoubleRow k-tile layout: [9, pair*1024 + plane*512 + e]
        ea8 = np.zeros((9, NP * 1024), np.float32)
        dloc = np.full((P, NP * 4), 999.0, np.float32)
        for k, (nodes, e0, e1) in enumerate(chunks):
            p, ci = divmod(k, 2)
            m = e1 - e0
            base = p * 1024 + ci * EPC
            sl = slice(base, base + m)
            xsd[:32, sl] = xf[src_s[e0:e1]].T
            xsd[32, sl] = 1.0
            sl2 = slice(base + 512, base + 512 + m)
            xsd[:32, sl2] = xf[dst_s[e0:e1]].T
            xsd[32, sl2] = 1.0
            eat = ea_s[e0:e1].T                       # [16, m]
            ea8[0:9, sl] = eat[0:9]
            ea8[0:7, sl2] = eat[9:16]
            ea8[7, sl2] = 1.0                         # aug ones row (=row 16)
            # slot index of each edge's dst within the chunk node list
            node_arr = np.asarray(nodes)
            slot_of = {n: i for i, n in enumerate(nodes)}
            dl = np.array([slot_of[n] for n in dst_s[e0:e1]], np.float32)
            # dloc columns: pair p has 4 subtiles (ci*2 + sub)
            full = np.full(EPC, 999.0, np.float32)
            full[:m] = dl
            dloc[:, p * 4 + ci * 2] = full[:P]
            dloc[:, p * 4 + ci * 2 + 1] = full[P:]
        in_maps.append({
            "xsd": xsd.astype(BF_NP),
            "ea8": ea8.astype(F8_NP),
            "dloc": dloc,
        })

    meta = dict(NP=NP, cores=cores, bounds=bounds)
    return in_maps, meta


# ----------------------------------------------------------------------------
# device kernel builder
# ----------------------------------------------------------------------------

def _build(NP):
    nc = bacc.Bacc("TRN2", target_bir_lowering=False, debug=False,
                   num_devices=N_CORES)

    xsd_d = nc.declare_dram_parameter("xsd", [33, NP * 1024], BF, isOutput=False)
    ea8_d = nc.declare_dram_parameter("ea8", [9, NP * 1024], F8, isOutput=False)
    dloc_d = nc.declare_dram_parameter("dloc", [P, NP * 4], FP, isOutput=False)
    out_d = nc.declare_dram_parameter("out", [NP * 2 * P, 24], FP, isOutput=True)

    wencA_d = nc.declare_dram_parameter("wencA", [33, P], FP, isOutput=False)
    sgm_d = nc.declare_dram_parameter("sgm", [P, 4], FP, isOutput=False)
    wfold_d = nc.declare_dram_parameter("wfold", [P, 96], FP, isOutput=False)
    wlr8_d = nc.declare_dram_parameter("wlr8", [P, 1024], F8, isOutput=False)
    we28_d = nc.declare_dram_parameter("we28", [9, 1024], F8, isOutput=False)
    wlra8_d = nc.declare_dram_parameter("wlra8", [P, 224], F8, isOutput=False)
    wea8_d = nc.declare_dram_parameter("wea8", [9, 224], F8, isOutput=False)

    with tile.TileContext(nc) as tc:
        with (
            tc.tile_pool(name="const", bufs=1) as constp,
            tc.tile_pool(name="gath", bufs=3) as gathp,
            tc.tile_pool(name="hsp", bufs=2) as hsp,
            tc.tile_pool(name="apool", bufs=2) as apool,
            tc.tile_pool(name="expool", bufs=2) as expool,
            tc.tile_pool(name="spool", bufs=2) as spool,
            tc.tile_pool(name="gfp", bufs=2) as gfp,
            tc.tile_pool(name="finp", bufs=2) as finp,
            tc.tile_pool(name="pph", bufs=2, space="PSUM") as pph,
            tc.tile_pool(name="pxp", bufs=2, space="PSUM") as pxp,
            tc.tile_pool(name="pdq", bufs=1, space="PSUM") as pdq,
            tc.tile_pool(name="ptg", bufs=1, space="PSUM") as ptg,
            tc.tile_pool(name="ptps", bufs=1, space="PSUM") as ptps,
            tc.tile_pool(name="pacc", bufs=1, space="PSUM") as pacc,
        ):
            def load_const_bf(dram, shape, nm):
                t = constp.tile(shape, BF, tag=nm, name=nm)
                nc.gpsimd.dma_start(out=t[:], in_=dram[:])
                return t

            def load_const_f8(dram, shape, nm):
                t = constp.tile(shape, F8, tag=nm, name=nm)
                nc.gpsimd.dma_start(out=t[:], in_=dram[:])
                return t

            wencA = load_const_bf(wencA_d, [33, P], "wencA")
            sgm = load_const_bf(sgm_d, [P, 4], "sgm")
            wfold = load_const_bf(wfold_d, [P, 96], "wfold")
            wlr8 = load_const_f8(wlr8_d, [P, 1024], "wlr8")
            we28 = load_const_f8(we28_d, [9, 1024], "we28")
            wlra8 = load_const_f8(wlra8_d, [P, 224], "wlra8")
            wea8 = load_const_f8(wea8_d, [9, 224], "wea8")

            dlocR = constp.tile([P, NP * 4], FP, name="dlocR")
            nc.sync.dma_start(out=dlocR[:], in_=dloc_d[:])

            ident = constp.tile([P, P], BF, name="ident")
            make_identity(nc, ident[:])
            iota_i = constp.tile([P, P], I32, name="iota_i")
            nc.gpsimd.iota(iota_i[:], pattern=[[1, P]], base=0,
                           channel_multiplier=0)
            iota_f = constp.tile([P, P], FP, name="iota_f")
            nc.vector.tensor_copy(iota_f[:], iota_i[:])

            state = {}

            def front(p):
                gx = gathp.tile([33, 1024], BF, tag="gx", name=f"gx_{p}")
                nc.gpsimd.dma_start(out=gx[:], in_=xsd_d[:, p * 1024:(p + 1) * 1024])
                ge8 = gathp.tile([9, 1024], F8, tag="ge8", name=f"ge8_{p}")
                nc.gpsimd.dma_start(out=ge8[:], in_=ea8_d[:, p * 1024:(p + 1) * 1024])
                ge8v = ge8[:].rearrange("p (t e) -> p t e", t=2)

                # encoder
                ph_s = pph.tile([P, 512], FP, tag="ph", name=f"phs_{p}")
                nc.tensor.matmul(ph_s[:], lhsT=wencA[:], rhs=gx[:, 0:512],
                                 start=True, stop=True)
                hs = hsp.tile([P, 512], BF, tag="hs", name=f"hs_{p}")
                nc.scalar.activation(hs[:], ph_s[:],
                                     mybir.ActivationFunctionType.Relu)
                # fp8 copies for the DoubleRow logit path (plane0=hs, plane1=hd)
                h8 = hsp.tile([P, 2, 512], F8, tag="h8", name=f"h8_{p}")
                nc.scalar.activation(h8[:, 0, :], ph_s[:],
                                     mybir.ActivationFunctionType.Relu)
                ph_d = pph.tile([P, 512], FP, tag="ph", name=f"phd_{p}")
                nc.tensor.matmul(ph_d[:], lhsT=wencA[:], rhs=gx[:, 512:1024],
                                 start=True, stop=True)
                nc.vector.tensor_scalar(out=h8[:, 1, :], in0=ph_d[:], scalar1=0.0,
                                        scalar2=None, op0=mybir.AluOpType.max)

                # X^T per head (DoubleRow fp8) + abs
                A = apool.tile([P, 2048], BF, tag="A", name=f"A_{p}")
                for h in range(4):
                    px = pxp.tile([P, 512], FP, tag="px", name=f"px_{p}_{h}")
                    wlr_h = wlr8[:, h * 256:(h + 1) * 256].rearrange(
                        "p (t c) -> p t c", t=2)
                    nc.tensor.matmul(px[:], lhsT=wlr_h, rhs=h8[:],
                                     start=True, stop=False, perf_mode=DR)
                    we2_h = we28[:, h * 256:(h + 1) * 256].rearrange(
                        "p (t c) -> p t c", t=2)
                    nc.tensor.matmul(px[:], lhsT=we2_h, rhs=ge8v,
                                     start=False, stop=True, perf_mode=DR)
                    asl = A[:, h * 512:(h + 1) * 512]
                    nc.scalar.activation(asl, px[:],
                                         mybir.ActivationFunctionType.Abs)

                # logits: t1 (partitions 32h) + signed abs sums  (64x scaled)
                dfq = pdq.tile([P, 512], FP, tag="dfq", name=f"dfq_{p}")
                nc.tensor.matmul(dfq[0:112, :],
                                 lhsT=wlra8[:].rearrange("p (t c) -> p t c", t=2),
                                 rhs=h8[:], start=True, stop=False, perf_mode=DR)
                nc.tensor.matmul(dfq[0:112, :],
                                 lhsT=wea8[:].rearrange("p (t c) -> p t c", t=2),
                                 rhs=ge8v, start=False, stop=False, perf_mode=DR)
                for h in range(4):
                    nc.tensor.matmul(dfq[32 * h:32 * h + 1, :],
                                     lhsT=sgm[:, h:h + 1],
                                     rhs=A[:, h * 512:(h + 1) * 512],
                                     start=False, stop=(h == 3),
                                     skip_group_check=True,
                                     tile_position=(0, 32 * h))

                ex = expool.tile([P, 512], BF, tag="ex", name=f"ex_{p}")
                nc.scalar.activation(ex[0:97, :], dfq[0:97, :],
                                     mybir.ActivationFunctionType.Exp,
                                     scale=1.0 / 64.0)

                # S indicator (DVE; Pool lacks TensorTensor on TRN2)
                S = spool.tile([P, 512], BF, tag="S", name=f"S_{p}")
                nc.vector.tensor_tensor(
                    out=S[:].rearrange("p (s n) -> p s n", s=4),
                    in0=dlocR[:, 4 * p:4 * p + 4].to_broadcast([P, 4, P]),
                    in1=iota_f[:].unsqueeze(1).to_broadcast([P, 4, P]),
                    op=mybir.AluOpType.is_equal)

                state[p] = dict(hs=hs, ex=ex, S=S)

            def tail(p):
                st = state.pop(p)
                hs, ex, S = st["hs"], st["ex"], st["S"]

                # ex transposed back to edge-major: tps[:, s, 32h] = ex_h
                # (slots padded to 100 cols => 200B, 4-byte aligned)
                tps = ptps.tile([P, 4, 100], BF, tag="tps", name=f"tps_{p}")
                for s in range(4):
                    nc.tensor.transpose(tps[:, s, 0:97],
                                        ex[0:97, s * P:(s + 1) * P],
                                        ident[0:97, 0:97])

                # gfold
                tg = ptg.tile([P, 4, 96], FP, tag="tg", name=f"tg_{p}")
                for s in range(4):
                    nc.tensor.matmul(tg[:, s, :],
                                     lhsT=hs[:, s * P:(s + 1) * P],
                                     rhs=wfold[:], start=True, stop=True)

                # gf = [gfold * ex | ex]; ex lands in SBUF first so the
                # multiply reads only one PSUM operand (tg)
                gf = gfp.tile([P, 4, 100], BF, tag="gf", name=f"gf_{p}")
                nc.vector.tensor_copy(gf[:, :, 96:100], tps[:, :, 0:97:32])
                exb = gf[:, :, 96:100].unsqueeze(3).to_broadcast([P, 4, 4, 24])
                nc.vector.tensor_tensor(
                    out=gf[:, :, 0:96].rearrange("p s (h j) -> p s h j", h=4),
                    in0=tg[:].rearrange("p s (h j) -> p s h j", h=4),
                    in1=exb, op=mybir.AluOpType.mult)

                # aggregate per subtile into per-chunk acc
                acc = pacc.tile([P, 2, 100], FP, tag="acc", name=f"acc_{p}")
                for s in range(4):
                    nc.tensor.matmul(acc[:, s // 2, :],
                                     lhsT=S[:, s * P:(s + 1) * P],
                                     rhs=gf[:, s, :],
                                     start=(s % 2 == 0), stop=(s % 2 == 1))

                # finalize both chunks
                srec = finp.tile([P, 2, 4], FP, tag="srec", name=f"srec_{p}")
                nc.vector.tensor_scalar_add(srec[:], acc[:, :, 96:100], 1e-6)
                rec = finp.tile([P, 2, 4], FP, tag="rec", name=f"rec_{p}")
                nc.vector.reciprocal(rec[:], srec[:])
                os = finp.tile([P, 2, 24, 4], FP, tag="os", name=f"os_{p}")
                nc.vector.tensor_tensor(
                    out=os[:].rearrange("p c j h -> p c h j"),
                    in0=acc[:, :, 0:96].rearrange("p c (h j) -> p c h j", h=4),
                    in1=rec[:].unsqueeze(3).to_broadcast([P, 2, 4, 24]),
                    op=mybir.AluOpType.mult)
                o2 = finp.tile([P, 48], FP, tag="o2", name=f"o2_{p}")
                nc.vector.tensor_reduce(
                    out=o2[:].rearrange("p (c j) -> p c j", c=2),
                    in_=os[:], axis=mybir.AxisListType.X,
                    op=mybir.AluOpType.add)
                for ci in range(2):
                    k = 2 * p + ci
                    nc.sync.dma_start(out=out_d[k * P:(k + 1) * P, :],
                                      in_=o2[:, ci * 24:(ci + 1) * 24])

            for p in range(NP):
                front(p)
                if p >= 1:
                    tail(p - 1)
            tail(NP - 1)

    nc.compile()
    return nc


# ----------------------------------------------------------------------------
# public entry
# ----------------------------------------------------------------------------

_CACHE = {}
LAST_RUN = {}


def kernel(**inputs):
    x = np.asarray(inputs["x"])
    edge_attr = np.asarray(inputs["edge_attr"])
    edge_index = np.asarray(inputs["edge_index"])
    fw = _fold_weights(
        *[np.asarray(inputs[k], np.float32) for k in
          ("W_enc", "b_enc", "bn_gamma", "bn_beta", "bn_mean", "bn_var",
           "Wl", "bl", "Wr", "br", "We", "att", "bias_conv", "Wp", "bp")])

    in_maps, meta = _prepare(x, edge_attr, edge_index)
    NP = meta["NP"]

    if NP not in _CACHE:
        _CACHE[NP] = _build(NP)
    nc = _CACHE[NP]

    wmap = {
        "wencA": fw["wencA"].astype(np.float32),
        "sgm": fw["sgm"].astype(np.float32),
        "wfold": fw["wfold"].astype(np.float32),
        "wlr8": fw["wlr8"].astype(F8_NP),
        "we28": fw["we28"].astype(F8_NP),
        "wlra8": fw["wlra8"].astype(F8_NP),
        "wea8": fw["wea8"].astype(F8_NP),
    }
    for im in in_maps:
        im.update(wmap)

    LAST_RUN["in_maps"] = in_maps
    LAST_RUN["nc"] = nc
    res = run_bass_kernel_spmd(nc, in_maps, core_ids=list(range(N_CORES)))

    # unshard
    N = x.shape[0]
    OUT = fw["OUT"]
    out = np.zeros((N, OUT), dtype=np.float32)
    for c, cd in enumerate(meta["cores"]):
        dev = np.asarray(res.results[c]["out"], np.float32)   # [NP*256, 24]
        for k, (nodes, e0, e1) in enumerate(cd["chunks"]):
            out[np.asarray(nodes)] = dev[k * P:k * P + len(nodes)]

    deg = np.bincount(np.asarray(edge_index[1], np.int64), minlength=N)
    sgn = (deg > 0).astype(np.float32)[:, None]
    out = out + sgn * fw["cbl"][None, :] + fw["cc"][None, :]
    return out.astype(np.float32)


# revision 23
# speedup vs baseline: 1.5255x; 1.5255x over previous
"""GATv2 molecular-graph kernel for 8 TRN2 NeuronCores (SPMD, data-parallel).

Host side (layout only — all reference FLOPs run on device): edges are
sorted by destination node and partitioned into 8 contiguous ranges with
~equal edge counts.  Per core, edges are packed into chunks of <=256 edge
slots covering <=128 distinct destination nodes; two chunks form a "pair"
(512 edge slots) which is the device work unit.  For each edge slot the
host gathers the RAW inputs (x[src], x[dst], edge_attr) into dense bf16
tensors, so the device needs no indirect DMA at all.

Device pipeline per pair (transposed feature-major layout):
  hsT/hdT = relu(wencA^T @ xsdT)            encoder (bn folded, bias via
                                            appended all-ones feature row)
  X^T_h   = wl2_h^T hs + wr2_h^T hd + we2_h^T ea    [C=128, 512 edges]
            (weights pre-scaled by 0.4*|att|)
  A_h     = |X^T_h|                          (scalar/vector engines)
  dfq     = t1 (3 matmuls, landing on PSUM partitions 0/32/64/96)
            + sum_c sign(att_c)*A_h  (4 single-column sigma-matmuls)
            = full GATv2 logits: 0.6*(z@att) + 0.4*sum|z_c att_c|
  ex      = exp(dfq)  (no segment-max shift; logits are O(0.1))
  gf      = [gfold*ex | ex]  where gfold = hs @ (Wl_h @ Wp_h)  [e,96]
  acc     = S^T @ gf  per subtile (S = slot-indicator, built on DVE)
  out     = sum_h acc_h * 1/(denom_h)       (vector engine)
Softmax denominators ride the last 4 columns of gf.  Host adds the
constant terms (bias_conv@Wp + bp, and bl@Wp gated by deg>0).
"""

import numpy as np
import ml_dtypes

import concourse.bacc as bacc
import concourse.tile as tile
from concourse import mybir
import concourse.bass as bass
from concourse.bass_utils import run_bass_kernel_spmd
from concourse.masks import make_identity

P = 128
N_CORES = 8
EPC = 256            # edge slots per chunk
EPP = 512            # edge slots per pair (2 chunks)
SLOTS = 128          # max distinct dst nodes per chunk
NEG_SLOPE = 0.2
BN_EPS = 1e-5

FP = mybir.dt.float32
BF = mybir.dt.bfloat16
I32 = mybir.dt.int32
BF_NP = ml_dtypes.bfloat16


# ----------------------------------------------------------------------------
# host-side weight folding
# ----------------------------------------------------------------------------

def _fold_weights(W_enc, b_enc, bn_gamma, bn_beta, bn_mean, bn_var,
                  Wl, bl, Wr, br, We, att, bias_conv, Wp, bp):
    D = W_enc.shape[1]
    H, C = att.shape
    HC = H * C
    OUT = Wp.shape[1]
    s = bn_gamma / np.sqrt(bn_var + BN_EPS)
    W_enc_f = W_enc * s[None, :]
    b_enc_f = (b_enc - bn_mean) * s + bn_beta
    wencA = np.concatenate([W_enc_f, b_enc_f[None, :]], 0)      # [33, D]

    att_abs = np.abs(att)                                       # [H, C]
    bsum = bl + br                                              # [HC]

    # abs-path weights: X = 0.4 * |att| * z   (z = hs@Wl + hd@Wr + ea@We + b)
    wl2 = np.zeros((D, HC), np.float32)
    wr2 = np.zeros((D, HC), np.float32)
    we2 = np.zeros((17, HC), np.float32)
    for h in range(H):
        blk = slice(h * C, (h + 1) * C)
        wl2[:, blk] = 0.4 * Wl[:, blk] * att_abs[h][None, :]
        wr2[:, blk] = 0.4 * Wr[:, blk] * att_abs[h][None, :]
        we2[:16, blk] = 0.4 * We[:, blk] * att_abs[h][None, :]
        we2[16, blk] = 0.4 * bsum[blk] * att_abs[h]

    sgm = np.where(att > 0, 1.0, -1.0).astype(np.float32).T     # [C, H]

    # linear logit path: t1 = 0.6 * (z @ att_h), landing on partition 32h
    wla = np.zeros((D, 97), np.float32)
    wra = np.zeros((D, 97), np.float32)
    wea = np.zeros((17, 97), np.float32)
    for h in range(H):
        blk = slice(h * C, (h + 1) * C)
        wla[:, 32 * h] = 0.6 * (Wl[:, blk] @ att[h])
        wra[:, 32 * h] = 0.6 * (Wr[:, blk] @ att[h])
        wea[:16, 32 * h] = 0.6 * (We[:, blk] @ att[h])
        wea[16, 32 * h] = 0.6 * (bsum[blk] @ att[h])

    # folded node->out transform per head
    wfold = np.concatenate(
        [Wl[:, h * C:(h + 1) * C] @ Wp[h * C:(h + 1) * C] for h in range(H)], 1
    )                                                           # [D, 96]

    cbl = bl @ Wp                       # [OUT]: * 1{deg>0}
    cc = bias_conv @ Wp + bp            # [OUT]: always
    return dict(wencA=wencA, wl2=wl2, wr2=wr2, we2=we2, sgm=sgm,
                wla=wla, wra=wra, wea=wea, wfold=wfold,
                cbl=cbl, cc=cc, H=H, C=C, OUT=OUT, D=D)


# ----------------------------------------------------------------------------
# host-side edge packing
# ----------------------------------------------------------------------------

def _prepare(x, edge_attr, edge_index):
    N = x.shape[0]
    E = edge_index.shape[1]
    src = np.asarray(edge_index[0], dtype=np.int64)
    dst = np.asarray(edge_index[1], dtype=np.int64)

    order = np.argsort(dst, kind="stable")
    src_s = src[order]
    dst_s = dst[order]
    ea_s = np.asarray(edge_attr, dtype=np.float32)[order]

    deg = np.bincount(dst, minlength=N)
    cum = np.concatenate([[0], np.cumsum(deg)])

    bounds = [0]
    for c in range(1, N_CORES):
        bounds.append(int(np.searchsorted(cum, E * c // N_CORES)))
    bounds.append(N)

    xf = np.asarray(x, dtype=np.float32)

    cores = []
    for c in range(N_CORES):
        n0, n1 = bounds[c], bounds[c + 1]
        # chunking: walk deg>0 nodes; <=SLOTS nodes and <=EPC edges per chunk
        chunks = []          # list of (list-of-node-ids, e_start, e_end)
        cur_nodes = []
        ce0 = int(cum[n0])
        ecnt = 0
        for n in range(n0, n1):
            d = int(deg[n])
            if d == 0:
                continue
            if d > EPC:
                raise RuntimeError("node degree exceeds chunk capacity")
            if len(cur_nodes) >= SLOTS or ecnt + d > EPC:
                chunks.append((cur_nodes, ce0, ce0 + ecnt))
                ce0 += ecnt
                cur_nodes = []
                ecnt = 0
            cur_nodes.append(n)
            ecnt += d
        if cur_nodes:
            chunks.append((cur_nodes, ce0, ce0 + ecnt))
        cores.append(dict(chunks=chunks, n0=n0, n1=n1))

    NCH = max(len(cd["chunks"]) for cd in cores)
    NP = (NCH + 1) // 2

    in_maps = []
    for cd in cores:
        chunks = cd["chunks"]
        # layout per pair p: [src slots (512) | dst slots (512)]
        xsd = np.zeros((33, NP * 1024), np.float32)
        ea17 = np.zeros((17, NP * EPP), np.float32)
        dloc = np.full((P, NP * 4), 999.0, np.float32)
        for k, (nodes, e0, e1) in enumerate(chunks):
            p, ci = divmod(k, 2)
            m = e1 - e0
            base = p * 1024 + ci * EPC
            sl = slice(base, base + m)
            xsd[:32, sl] = xf[src_s[e0:e1]].T
            xsd[32, sl] = 1.0
            sl2 = slice(base + 512, base + 512 + m)
            xsd[:32, sl2] = xf[dst_s[e0:e1]].T
            xsd[32, sl2] = 1.0
            eb = p * EPP + ci * EPC
            ea17[:16, eb:eb + m] = ea_s[e0:e1].T
            ea17[16, eb:eb + m] = 1.0
            # slot index of each edge's dst within the chunk node list
            slot_of = {n: i for i, n in enumerate(nodes)}
            dl = np.array([slot_of[n] for n in dst_s[e0:e1]], np.float32)
            full = np.full(EPC, 999.0, np.float32)
            full[:m] = dl
            dloc[:, p * 4 + ci * 2] = full[:P]
            dloc[:, p * 4 + ci * 2 + 1] = full[P:]
        in_maps.append({
            "xsd": xsd.astype(BF_NP),
            "ea17": ea17.astype(BF_NP),
            "dloc": dloc,
        })

    meta = dict(NP=NP, cores=cores, bounds=bounds)
    return in_maps, meta


# ----------------------------------------------------------------------------
# device kernel builder
# ----------------------------------------------------------------------------

def _build(NP):
    nc = bacc.Bacc("TRN2", target_bir_lowering=False, debug=False,
                   num_devices=N_CORES)

    xsd_d = nc.declare_dram_parameter("xsd", [33, NP * 1024], BF, isOutput=False)
    ea_d = nc.declare_dram_parameter("ea17", [17, NP * EPP], BF, isOutput=False)
    dloc_d = nc.declare_dram_parameter("dloc", [P, NP * 4], FP, isOutput=False)
    out_d = nc.declare_dram_parameter("out", [NP * 2 * P, 24], FP, isOutput=True)

    wencA_d = nc.declare_dram_parameter("wencA", [33, P], FP, isOutput=False)
    wl2_d = nc.declare_dram_parameter("wl2", [P, 512], FP, isOutput=False)
    wr2_d = nc.declare_dram_parameter("wr2", [P, 512], FP, isOutput=False)
    we2_d = nc.declare_dram_parameter("we2", [17, 512], FP, isOutput=False)
    sgm_d = nc.declare_dram_parameter("sgm", [P, 4], FP, isOutput=False)
    wla_d = nc.declare_dram_parameter("wla", [P, 97], FP, isOutput=False)
    wra_d = nc.declare_dram_parameter("wra", [P, 97], FP, isOutput=False)
    wea_d = nc.declare_dram_parameter("wea", [17, 97], FP, isOutput=False)
    wfold_d = nc.declare_dram_parameter("wfold", [P, 96], FP, isOutput=False)

    with tile.TileContext(nc) as tc:
        with (
            tc.tile_pool(name="const", bufs=1) as constp,
            tc.tile_pool(name="gath", bufs=3) as gathp,
            tc.tile_pool(name="hsp", bufs=2) as hsp,
            tc.tile_pool(name="apool", bufs=2) as apool,
            tc.tile_pool(name="expool", bufs=2) as expool,
            tc.tile_pool(name="spool", bufs=2) as spool,
            tc.tile_pool(name="gfp", bufs=2) as gfp,
            tc.tile_pool(name="finp", bufs=2) as finp,
            tc.tile_pool(name="pph", bufs=2, space="PSUM") as pph,
            tc.tile_pool(name="pxp", bufs=2, space="PSUM") as pxp,
            tc.tile_pool(name="pdq", bufs=1, space="PSUM") as pdq,
            tc.tile_pool(name="ptg", bufs=1, space="PSUM") as ptg,
            tc.tile_pool(name="ptps", bufs=1, space="PSUM") as ptps,
            tc.tile_pool(name="pacc", bufs=1, space="PSUM") as pacc,
        ):
            def load_const_bf(dram, shape, nm):
                t = constp.tile(shape, BF, tag=nm, name=nm)
                nc.gpsimd.dma_start(out=t[:], in_=dram[:])
                return t

            wencA = load_const_bf(wencA_d, [33, P], "wencA")
            wl2 = load_const_bf(wl2_d, [P, 512], "wl2")
            wr2 = load_const_bf(wr2_d, [P, 512], "wr2")
            we2 = load_const_bf(we2_d, [17, 512], "we2")
            sgm = load_const_bf(sgm_d, [P, 4], "sgm")
            wla = load_const_bf(wla_d, [P, 97], "wla")
            wra = load_const_bf(wra_d, [P, 97], "wra")
            wea = load_const_bf(wea_d, [17, 97], "wea")
            wfold = load_const_bf(wfold_d, [P, 96], "wfold")

            dlocR = constp.tile([P, NP * 4], FP, name="dlocR")
            nc.sync.dma_start(out=dlocR[:], in_=dloc_d[:])

            ident = constp.tile([P, P], BF, name="ident")
            make_identity(nc, ident[:])
            iota_i = constp.tile([P, P], I32, name="iota_i")
            nc.gpsimd.iota(iota_i[:], pattern=[[1, P]], base=0,
                           channel_multiplier=0)
            iota_f = constp.tile([P, P], FP, name="iota_f")
            nc.vector.tensor_copy(iota_f[:], iota_i[:])

            state = {}

            def front(p):
                gx = gathp.tile([33, 1024], BF, tag="gx", name=f"gx_{p}")
                nc.gpsimd.dma_start(out=gx[:], in_=xsd_d[:, p * 1024:(p + 1) * 1024])
                ge = gathp.tile([17, EPP], BF, tag="ge", name=f"ge_{p}")
                nc.gpsimd.dma_start(out=ge[:], in_=ea_d[:, p * EPP:(p + 1) * EPP])

                # encoder
                ph_s = pph.tile([P, 512], FP, tag="ph", name=f"phs_{p}")
                nc.tensor.matmul(ph_s[:], lhsT=wencA[:], rhs=gx[:, 0:512],
                                 start=True, stop=True)
                hs = hsp.tile([P, 512], BF, tag="hs", name=f"hs_{p}")
                nc.scalar.activation(hs[:], ph_s[:],
                                     mybir.ActivationFunctionType.Relu)
                ph_d = pph.tile([P, 512], FP, tag="ph", name=f"phd_{p}")
                nc.tensor.matmul(ph_d[:], lhsT=wencA[:], rhs=gx[:, 512:1024],
                                 start=True, stop=True)
                hd = hsp.tile([P, 512], BF, tag="hd", name=f"hd_{p}")
                nc.vector.tensor_scalar(out=hd[:], in0=ph_d[:], scalar1=0.0,
                                        scalar2=None, op0=mybir.AluOpType.max)

                # X^T per head + abs (abs h3 on DVE to shorten the ACT tail)
                A = apool.tile([P, 2048], BF, tag="A", name=f"A_{p}")
                for h in range(4):
                    px = pxp.tile([P, 512], FP, tag="px", name=f"px_{p}_{h}")
                    nc.tensor.matmul(px[:], lhsT=wl2[:, h * P:(h + 1) * P],
                                     rhs=hs[:], start=True, stop=False)
                    nc.tensor.matmul(px[:], lhsT=wr2[:, h * P:(h + 1) * P],
                                     rhs=hd[:], start=False, stop=False)
                    nc.tensor.matmul(px[:], lhsT=we2[:, h * P:(h + 1) * P],
                                     rhs=ge[:], start=False, stop=True)
                    asl = A[:, h * 512:(h + 1) * 512]
                    if h == 3:
                        with nc.allow_low_precision(reason="singleton reduce"):
                            nc.vector.tensor_reduce(
                                out=asl, in_=px[:].unsqueeze(2),
                                axis=mybir.AxisListType.X,
                                op=mybir.AluOpType.add,
                                apply_absolute_value=True)
                    else:
                        nc.scalar.activation(asl, px[:],
                                             mybir.ActivationFunctionType.Abs)

                # logits: t1 (partitions 32h) + signed abs sums
                dfq = pdq.tile([P, 512], FP, tag="dfq", name=f"dfq_{p}")
                nc.tensor.matmul(dfq[0:97, :], lhsT=wla[:], rhs=hs[:],
                                 start=True, stop=False)
                nc.tensor.matmul(dfq[0:97, :], lhsT=wra[:], rhs=hd[:],
                                 start=False, stop=False)
                nc.tensor.matmul(dfq[0:97, :], lhsT=wea[:], rhs=ge[:],
                                 start=False, stop=False)
                for h in range(4):
                    nc.tensor.matmul(dfq[32 * h:32 * h + 1, :],
                                     lhsT=sgm[:, h:h + 1],
                                     rhs=A[:, h * 512:(h + 1) * 512],
                                     start=False, stop=(h == 3),
                                     skip_group_check=True,
                                     tile_position=(0, 32 * h))

                ex = expool.tile([P, 512], BF, tag="ex", name=f"ex_{p}")
                nc.scalar.activation(ex[0:97, :], dfq[0:97, :],
                                     mybir.ActivationFunctionType.Exp)

                # S indicator
                S = spool.tile([P, 512], BF, tag="S", name=f"S_{p}")
                nc.vector.tensor_tensor(
                    out=S[:].rearrange("p (s n) -> p s n", s=4),
                    in0=dlocR[:, 4 * p:4 * p + 4].to_broadcast([P, 4, P]),
                    in1=iota_f[:].unsqueeze(1).to_broadcast([P, 4, P]),
                    op=mybir.AluOpType.is_equal)

                state[p] = dict(hs=hs, ex=ex, S=S)

            def tail(p):
                st = state.pop(p)
                hs, ex, S = st["hs"], st["ex"], st["S"]

                # ex transposed back to edge-major: tps[:, s, 32h] = ex_h
                tps = ptps.tile([P, 4, 100], BF, tag="tps", name=f"tps_{p}")
                for s in range(4):
                    nc.tensor.transpose(tps[:, s, 0:97],
                                        ex[0:97, s * P:(s + 1) * P],
                                        ident[0:97, 0:97])

                # gfold
                tg = ptg.tile([P, 4, 96], FP, tag="tg", name=f"tg_{p}")
                for s in range(4):
                    nc.tensor.matmul(tg[:, s, :],
                                     lhsT=hs[:, s * P:(s + 1) * P],
                                     rhs=wfold[:], start=True, stop=True)

                # gf = [gfold * ex | ex]; ex lands in SBUF first so the
                # multiply reads only one PSUM operand (tg)
                gf = gfp.tile([P, 4, 100], BF, tag="gf", name=f"gf_{p}")
                nc.vector.tensor_copy(gf[:, :, 96:100], tps[:, :, 0:97:32])
                exb = gf[:, :, 96:100].unsqueeze(3).to_broadcast([P, 4, 4, 24])
                nc.vector.tensor_tensor(
                    out=gf[:, :, 0:96].rearrange("p s (h j) -> p s h j", h=4),
                    in0=tg[:].rearrange("p s (h j) -> p s h j", h=4),
                    in1=exb, op=mybir.AluOpType.mult)

                # aggregate per subtile into per-chunk acc
                acc = pacc.tile([P, 2, 100], FP, tag="acc", name=f"acc_{p}")
                for s in range(4):
                    nc.tensor.matmul(acc[:, s // 2, :],
                                     lhsT=S[:, s * P:(s + 1) * P],
                                     rhs=gf[:, s, :],
                                     start=(s % 2 == 0), stop=(s % 2 == 1))

                # finalize both chunks
                srec = finp.tile([P, 2, 4], FP, tag="srec", name=f"srec_{p}")
                nc.vector.tensor_scalar_add(srec[:], acc[:, :, 96:100], 1e-6)
                rec = finp.tile([P, 2, 4], FP, tag="rec", name=f"rec_{p}")
                nc.vector.reciprocal(rec[:], srec[:])
                os = finp.tile([P, 2, 24, 4], FP, tag="os", name=f"os_{p}")
                nc.vector.tensor_tensor(
                    out=os[:].rearrange("p c j h -> p c h j"),
                    in0=acc[:, :, 0:96].rearrange("p c (h j) -> p c h j", h=4),
                    in1=rec[:].unsqueeze(3).to_broadcast([P, 2, 4, 24]),
                    op=mybir.AluOpType.mult)
                o2 = finp.tile([P, 48], FP, tag="o2", name=f"o2_{p}")
                nc.vector.tensor_reduce(
                    out=o2[:].rearrange("p (c j) -> p c j", c=2),
                    in_=os[:], axis=mybir.AxisListType.X,
                    op=mybir.AluOpType.add)
                for ci in range(2):
                    k = 2 * p + ci
                    nc.sync.dma_start(out=out_d[k * P:(k + 1) * P, :],
                                      in_=o2[:, ci * 24:(ci + 1) * 24])

            for p in range(NP):
                front(p)
                if p >= 1:
                    tail(p - 1)
            tail(NP - 1)

    nc.compile()
    return nc


# ----------------------------------------------------------------------------
# public entry
# ----------------------------------------------------------------------------

_CACHE = {}
LAST_RUN = {}


def kernel(**inputs):
    x = np.asarray(inputs["x"])
    edge_attr = np.asarray(inputs["edge_attr"])
    edge_index = np.asarray(inputs["edge_index"])
    fw = _fold_weights(
        *[np.asarray(inputs[k], np.float32) for k in
          ("W_enc", "b_enc", "bn_gamma", "bn_beta", "bn_mean", "bn_var",
           "Wl", "bl", "Wr", "br", "We", "att", "bias_conv", "Wp", "bp")])

    in_maps, meta = _prepare(x, edge_attr, edge_index)
    NP = meta["NP"]

    if NP not in _CACHE:
        _CACHE[NP] = _build(NP)
    nc = _CACHE[NP]

    wmap = {
        "wencA": fw["wencA"].astype(np.float32),
        "wl2": fw["wl2"].astype(np.float32),
        "wr2": fw["wr2"].astype(np.float32),
        "we2": fw["we2"].astype(np.float32),
        "sgm": fw["sgm"].astype(np.float32),
        "wla": fw["wla"].astype(np.float32),
        "wra": fw["wra"].astype(np.float32),
        "wea": fw["wea"].astype(np.float32),
        "wfold": fw["wfold"].astype(np.float32),
    }
    for im in in_maps:
        im.update(wmap)

    LAST_RUN["in_maps"] = in_maps
    LAST_RUN["nc"] = nc
    res = run_bass_kernel_spmd(nc, in_maps, core_ids=list(range(N_CORES)))

    # unshard
    N = x.shape[0]
    OUT = fw["OUT"]
    out = np.zeros((N, OUT), dtype=np.float32)
    for c, cd in enumerate(meta["cores"]):
        dev = np.asarray(res.results[c]["out"], np.float32)   # [NP*256, 24]
        for k, (nodes, e0, e1) in enumerate(cd["chunks"]):
            out[np.asarray(nodes)] = dev[k * P:k * P + len(nodes)]

    deg = np.bincount(np.asarray(edge_index[1], np.int64), minlength=N)
    sgn = (deg > 0).astype(np.float32)[:, None]
    out = out + sgn * fw["cbl"][None, :] + fw["cc"][None, :]
    return out.astype(np.float32)


# revision 29
# speedup vs baseline: 1.6446x; 1.0781x over previous
"""GATv2 molecular-graph kernel for 8 TRN2 NeuronCores (SPMD, data-parallel).

Host side (layout only — all reference FLOPs run on device): edges are
sorted by destination node and partitioned into 8 contiguous ranges with
~equal edge counts.  Per core, edges are packed into chunks of <=256 edge
slots covering <=128 distinct destination nodes; two chunks form a "pair"
(512 edge slots) which is the device work unit.  For each edge slot the
host gathers the RAW inputs (x[src], x[dst], edge_attr) into dense bf16
tensors, so the device needs no indirect DMA at all.

Device pipeline per pair (transposed feature-major layout):
  hsT/hdT = relu(wencA^T @ xsdT)            encoder (bn folded, bias via
                                            appended all-ones feature row)
  X^T_h   = wl2_h^T hs + wr2_h^T hd + we2_h^T ea    [C=128, 512 edges]
            (weights pre-scaled by 0.4*|att|)
  A_h     = |X^T_h|                          (scalar/vector engines)
  dfq     = t1 (3 matmuls, landing on PSUM partitions 0/32/64/96)
            + sum_c sign(att_c)*A_h  (4 single-column sigma-matmuls)
            = full GATv2 logits: 0.6*(z@att) + 0.4*sum|z_c att_c|
  ex      = exp(dfq)  (no segment-max shift; logits are O(0.1))
  gf      = [gfold*ex | ex]  where gfold = hs @ (Wl_h @ Wp_h)  [e,96]
  acc     = S^T @ gf  per subtile (S = slot-indicator, built on DVE)
  out     = sum_h acc_h * 1/(denom_h)       (vector engine)
Softmax denominators ride the last 4 columns of gf.  Host adds the
constant terms (bias_conv@Wp + bp, and bl@Wp gated by deg>0).
"""

import numpy as np
import ml_dtypes

import concourse.bacc as bacc
import concourse.tile as tile
from concourse import mybir
import concourse.bass as bass
from concourse.bass_utils import run_bass_kernel_spmd
from concourse.masks import make_identity

P = 128
N_CORES = 8
EPC = 256            # edge slots per chunk
EPP = 512            # edge slots per pair (2 chunks)
SLOTS = 128          # max distinct dst nodes per chunk
NEG_SLOPE = 0.2
BN_EPS = 1e-5

FP = mybir.dt.float32
BF = mybir.dt.bfloat16
I32 = mybir.dt.int32
BF_NP = ml_dtypes.bfloat16


# ----------------------------------------------------------------------------
# host-side weight folding
# ----------------------------------------------------------------------------

def _fold_weights(W_enc, b_enc, bn_gamma, bn_beta, bn_mean, bn_var,
                  Wl, bl, Wr, br, We, att, bias_conv, Wp, bp):
    D = W_enc.shape[1]
    H, C = att.shape
    HC = H * C
    OUT = Wp.shape[1]
    s = bn_gamma / np.sqrt(bn_var + BN_EPS)
    W_enc_f = W_enc * s[None, :]
    b_enc_f = (b_enc - bn_mean) * s + bn_beta
    wencA = np.concatenate([W_enc_f, b_enc_f[None, :]], 0)      # [33, D]

    att_abs = np.abs(att)                                       # [H, C]
    bsum = bl + br                                              # [HC]

    # abs-path weights: X = 0.4 * |att| * z   (z = hs@Wl + hd@Wr + ea@We + b)
    wl2 = np.zeros((D, HC), np.float32)
    wr2 = np.zeros((D, HC), np.float32)
    we2 = np.zeros((17, HC), np.float32)
    for h in range(H):
        blk = slice(h * C, (h + 1) * C)
        wl2[:, blk] = 0.4 * Wl[:, blk] * att_abs[h][None, :]
        wr2[:, blk] = 0.4 * Wr[:, blk] * att_abs[h][None, :]
        we2[:16, blk] = 0.4 * We[:, blk] * att_abs[h][None, :]
        we2[16, blk] = 0.4 * bsum[blk] * att_abs[h]

    sgm = np.where(att > 0, 1.0, -1.0).astype(np.float32).T     # [C, H]

    # linear logit path: t1 = 0.6 * (z @ att_h), landing on partition 32h
    wla = np.zeros((D, 97), np.float32)
    wra = np.zeros((D, 97), np.float32)
    wea = np.zeros((17, 97), np.float32)
    for h in range(H):
        blk = slice(h * C, (h + 1) * C)
        wla[:, 32 * h] = 0.6 * (Wl[:, blk] @ att[h])
        wra[:, 32 * h] = 0.6 * (Wr[:, blk] @ att[h])
        wea[:16, 32 * h] = 0.6 * (We[:, blk] @ att[h])
        wea[16, 32 * h] = 0.6 * (bsum[blk] @ att[h])

    # folded node->out transform per head
    wfold = np.concatenate(
        [Wl[:, h * C:(h + 1) * C] @ Wp[h * C:(h + 1) * C] for h in range(H)], 1
    )                                                           # [D, 96]

    cbl = bl @ Wp                       # [OUT]: * 1{deg>0}
    cc = bias_conv @ Wp + bp            # [OUT]: always
    sel4 = np.zeros((97, 4), np.float32)
    for h in range(H):
        sel4[32 * h, h] = 1.0
    return dict(sel4=sel4, wencA=wencA, wl2=wl2, wr2=wr2, we2=we2, sgm=sgm,
                wla=wla, wra=wra, wea=wea, wfold=wfold,
                cbl=cbl, cc=cc, H=H, C=C, OUT=OUT, D=D)


# ----------------------------------------------------------------------------
# host-side edge packing
# ----------------------------------------------------------------------------

def _prepare(x, edge_attr, edge_index):
    N = x.shape[0]
    E = edge_index.shape[1]
    src = np.asarray(edge_index[0], dtype=np.int64)
    dst = np.asarray(edge_index[1], dtype=np.int64)

    order = np.argsort(dst, kind="stable")
    src_s = src[order]
    dst_s = dst[order]
    ea_s = np.asarray(edge_attr, dtype=np.float32)[order]

    deg = np.bincount(dst, minlength=N)
    cum = np.concatenate([[0], np.cumsum(deg)])

    bounds = [0]
    for c in range(1, N_CORES):
        bounds.append(int(np.searchsorted(cum, E * c // N_CORES)))
    bounds.append(N)

    xf = np.asarray(x, dtype=np.float32)

    cores = []
    for c in range(N_CORES):
        n0, n1 = bounds[c], bounds[c + 1]
        # chunking: walk deg>0 nodes; <=SLOTS nodes and <=EPC edges per chunk
        chunks = []          # list of (list-of-node-ids, e_start, e_end)
        cur_nodes = []
        ce0 = int(cum[n0])
        ecnt = 0
        for n in range(n0, n1):
            d = int(deg[n])
            if d == 0:
                continue
            if d > EPC:
                raise RuntimeError("node degree exceeds chunk capacity")
            if len(cur_nodes) >= SLOTS or ecnt + d > EPC:
                chunks.append((cur_nodes, ce0, ce0 + ecnt))
                ce0 += ecnt
                cur_nodes = []
                ecnt = 0
            cur_nodes.append(n)
            ecnt += d
        if cur_nodes:
            chunks.append((cur_nodes, ce0, ce0 + ecnt))
        cores.append(dict(chunks=chunks, n0=n0, n1=n1))

    NCH = max(len(cd["chunks"]) for cd in cores)
    NP = (NCH + 1) // 2

    in_maps = []
    for cd in cores:
        chunks = cd["chunks"]
        # layout per pair p: [src slots (512) | dst slots (512)]
        xsd = np.zeros((33, NP * 1024), np.float32)
        ea17 = np.zeros((17, NP * EPP), np.float32)
        dloc = np.full((P, NP * 4), 999.0, np.float32)
        for k, (nodes, e0, e1) in enumerate(chunks):
            p, ci = divmod(k, 2)
            m = e1 - e0
            base = p * 1024 + ci * EPC
            sl = slice(base, base + m)
            xsd[:32, sl] = xf[src_s[e0:e1]].T
            xsd[32, sl] = 1.0
            sl2 = slice(base + 512, base + 512 + m)
            xsd[:32, sl2] = xf[dst_s[e0:e1]].T
            xsd[32, sl2] = 1.0
            eb = p * EPP + ci * EPC
            ea17[:16, eb:eb + m] = ea_s[e0:e1].T
            ea17[16, eb:eb + m] = 1.0
            # slot index of each edge's dst within the chunk node list
            slot_of = {n: i for i, n in enumerate(nodes)}
            dl = np.array([slot_of[n] for n in dst_s[e0:e1]], np.float32)
            full = np.full(EPC, 999.0, np.float32)
            full[:m] = dl
            dloc[:, p * 4 + ci * 2] = full[:P]
            dloc[:, p * 4 + ci * 2 + 1] = full[P:]
        in_maps.append({
            "xsd": xsd.astype(BF_NP),
            "ea17": ea17.astype(BF_NP),
            "dloc": dloc,
        })

    meta = dict(NP=NP, cores=cores, bounds=bounds)
    return in_maps, meta


# ----------------------------------------------------------------------------
# device kernel builder
# ----------------------------------------------------------------------------

def _build(NP):
    nc = bacc.Bacc("TRN2", target_bir_lowering=False, debug=False,
                   num_devices=N_CORES)

    xsd_d = nc.declare_dram_parameter("xsd", [33, NP * 1024], BF, isOutput=False)
    ea_d = nc.declare_dram_parameter("ea17", [17, NP * EPP], BF, isOutput=False)
    dloc_d = nc.declare_dram_parameter("dloc", [P, NP * 4], FP, isOutput=False)
    out_d = nc.declare_dram_parameter("out", [NP * 2 * P, 24], FP, isOutput=True)

    wencA_d = nc.declare_dram_parameter("wencA", [33, P], FP, isOutput=False)
    wl2_d = nc.declare_dram_parameter("wl2", [P, 512], FP, isOutput=False)
    wr2_d = nc.declare_dram_parameter("wr2", [P, 512], FP, isOutput=False)
    we2_d = nc.declare_dram_parameter("we2", [17, 512], FP, isOutput=False)
    sgm_d = nc.declare_dram_parameter("sgm", [P, 4], FP, isOutput=False)
    wla_d = nc.declare_dram_parameter("wla", [P, 97], FP, isOutput=False)
    wra_d = nc.declare_dram_parameter("wra", [P, 97], FP, isOutput=False)
    wea_d = nc.declare_dram_parameter("wea", [17, 97], FP, isOutput=False)
    wfold_d = nc.declare_dram_parameter("wfold", [P, 96], FP, isOutput=False)
    sel4_d = nc.declare_dram_parameter("sel4", [97, 4], FP, isOutput=False)

    with tile.TileContext(nc) as tc:
        with (
            tc.tile_pool(name="const", bufs=1) as constp,
            tc.tile_pool(name="gath", bufs=3) as gathp,
            tc.tile_pool(name="hsp", bufs=2) as hsp,
            tc.tile_pool(name="apool", bufs=2) as apool,
            tc.tile_pool(name="expool", bufs=2) as expool,
            tc.tile_pool(name="spool", bufs=2) as spool,
            tc.tile_pool(name="gfp", bufs=2) as gfp,
            tc.tile_pool(name="finp", bufs=2) as finp,
            tc.tile_pool(name="pph", bufs=2, space="PSUM") as pph,
            tc.tile_pool(name="pxp", bufs=2, space="PSUM") as pxp,
            tc.tile_pool(name="pdq", bufs=2, space="PSUM") as pdq,
            tc.tile_pool(name="ptg", bufs=1, space="PSUM") as ptg,
            tc.tile_pool(name="pacc", bufs=1, space="PSUM") as pacc,
        ):
            def load_const_bf(dram, shape, nm):
                t = constp.tile(shape, BF, tag=nm, name=nm)
                nc.gpsimd.dma_start(out=t[:], in_=dram[:])
                return t

            wencA = load_const_bf(wencA_d, [33, P], "wencA")
            wl2 = load_const_bf(wl2_d, [P, 512], "wl2")
            wr2 = load_const_bf(wr2_d, [P, 512], "wr2")
            we2 = load_const_bf(we2_d, [17, 512], "we2")
            sgm = load_const_bf(sgm_d, [P, 4], "sgm")
            wla = load_const_bf(wla_d, [P, 97], "wla")
            wra = load_const_bf(wra_d, [P, 97], "wra")
            wea = load_const_bf(wea_d, [17, 97], "wea")
            wfold = load_const_bf(wfold_d, [P, 96], "wfold")
            sel4 = load_const_bf(sel4_d, [97, 4], "sel4")

            dlocR = constp.tile([P, NP * 4], FP, name="dlocR")
            nc.sync.dma_start(out=dlocR[:], in_=dloc_d[:])

            iota_i = constp.tile([P, P], I32, name="iota_i")
            nc.gpsimd.iota(iota_i[:], pattern=[[1, P]], base=0,
                           channel_multiplier=0)
            iota_f = constp.tile([P, P], FP, name="iota_f")
            nc.vector.tensor_copy(iota_f[:], iota_i[:])

            state = {}

            def front(p):
                gx = gathp.tile([33, 1024], BF, tag="gx", name=f"gx_{p}")
                nc.gpsimd.dma_start(out=gx[:], in_=xsd_d[:, p * 1024:(p + 1) * 1024])
                ge = gathp.tile([17, EPP], BF, tag="ge", name=f"ge_{p}")
                nc.gpsimd.dma_start(out=ge[:], in_=ea_d[:, p * EPP:(p + 1) * EPP])

                # encoder
                ph_s = pph.tile([P, 512], FP, tag="ph", name=f"phs_{p}")
                nc.tensor.matmul(ph_s[:], lhsT=wencA[:], rhs=gx[:, 0:512],
                                 start=True, stop=True)
                hs = hsp.tile([P, 512], BF, tag="hs", name=f"hs_{p}")
                nc.scalar.activation(hs[:], ph_s[:],
                                     mybir.ActivationFunctionType.Relu)
                ph_d = pph.tile([P, 512], FP, tag="ph", name=f"phd_{p}")
                nc.tensor.matmul(ph_d[:], lhsT=wencA[:], rhs=gx[:, 512:1024],
                                 start=True, stop=True)
                hd = hsp.tile([P, 512], BF, tag="hd", name=f"hd_{p}")
                nc.vector.tensor_scalar(out=hd[:], in0=ph_d[:], scalar1=0.0,
                                        scalar2=None, op0=mybir.AluOpType.max)

                # X^T per head + abs (abs h3 on DVE to shorten the ACT tail)
                A = apool.tile([P, 2048], BF, tag="A", name=f"A_{p}")
                for h in range(4):
                    px = pxp.tile([P, 512], FP, tag="px", name=f"px_{p}_{h}")
                    nc.tensor.matmul(px[:], lhsT=wl2[:, h * P:(h + 1) * P],
                                     rhs=hs[:], start=True, stop=False)
                    nc.tensor.matmul(px[:], lhsT=wr2[:, h * P:(h + 1) * P],
                                     rhs=hd[:], start=False, stop=False)
                    nc.tensor.matmul(px[:], lhsT=we2[:, h * P:(h + 1) * P],
                                     rhs=ge[:], start=False, stop=True)
                    asl = A[:, h * 512:(h + 1) * 512]
                    nc.scalar.activation(asl, px[:],
                                         mybir.ActivationFunctionType.Abs)

                # logits: t1 (partitions 32h) + signed abs sums
                dfq = pdq.tile([P, 512], FP, tag="dfq", name=f"dfq_{p}")
                nc.tensor.matmul(dfq[0:97, :], lhsT=wla[:], rhs=hs[:],
                                 start=True, stop=False)
                nc.tensor.matmul(dfq[0:97, :], lhsT=wra[:], rhs=hd[:],
                                 start=False, stop=False)
                nc.tensor.matmul(dfq[0:97, :], lhsT=wea[:], rhs=ge[:],
                                 start=False, stop=False)
                for h in range(4):
                    nc.tensor.matmul(dfq[32 * h:32 * h + 1, :],
                                     lhsT=sgm[:, h:h + 1],
                                     rhs=A[:, h * 512:(h + 1) * 512],
                                     start=False, stop=(h == 3),
                                     skip_group_check=True,
                                     tile_position=(0, 32 * h))

                ex = expool.tile([P, 512], BF, tag="ex", name=f"ex_{p}")
                nc.scalar.activation(ex[0:97, :], dfq[0:97, :],
                                     mybir.ActivationFunctionType.Exp)

                # S indicator
                S = spool.tile([P, 512], BF, tag="S", name=f"S_{p}")
                nc.vector.tensor_tensor(
                    out=S[:].rearrange("p (s n) -> p s n", s=4),
                    in0=dlocR[:, 4 * p:4 * p + 4].to_broadcast([P, 4, P]),
                    in1=iota_f[:].unsqueeze(1).to_broadcast([P, 4, P]),
                    op=mybir.AluOpType.is_equal)

                state[p] = dict(hs=hs, ex=ex, S=S)

            def tail(p):
                st = state.pop(p)
                hs, ex, S = st["hs"], st["ex"], st["S"]

                # acc bank also hosts the compacted ex rows (cols 200:216)
                pm = pacc.tile([P, 216], FP, tag="acc", name=f"pm_{p}")
                acc = pm[:, 0:200].rearrange("p (c w) -> p c w", c=2)
                tps = pm[:, 200:216].rearrange("p (s h) -> p s h", s=4)

                # ex rows {0,32,64,96} back to edge-major via a selector
                # matmul: tps[e, s, h] = ex[32h, s*128+e]
                for s in range(4):
                    nc.tensor.matmul(tps[:, s, :],
                                     lhsT=ex[0:97, s * P:(s + 1) * P],
                                     rhs=sel4[:], start=True, stop=True,
                                     skip_group_check=True)

                # gfold
                tg = ptg.tile([P, 4, 96], FP, tag="tg", name=f"tg_{p}")
                for s in range(4):
                    nc.tensor.matmul(tg[:, s, :],
                                     lhsT=hs[:, s * P:(s + 1) * P],
                                     rhs=wfold[:], start=True, stop=True)

                # gf = [gfold * ex | ex]; ex lands in SBUF first so the
                # multiply reads only one PSUM operand (tg)
                gf = gfp.tile([P, 4, 100], BF, tag="gf", name=f"gf_{p}")
                nc.vector.tensor_copy(gf[:, :, 96:100], tps[:])
                exb = gf[:, :, 96:100].unsqueeze(3).to_broadcast([P, 4, 4, 24])
                nc.vector.tensor_tensor(
                    out=gf[:, :, 0:96].rearrange("p s (h j) -> p s h j", h=4),
                    in0=tg[:].rearrange("p s (h j) -> p s h j", h=4),
                    in1=exb, op=mybir.AluOpType.mult)

                # aggregate per subtile into per-chunk acc
                for s in range(4):
                    nc.tensor.matmul(acc[:, s // 2, :],
                                     lhsT=S[:, s * P:(s + 1) * P],
                                     rhs=gf[:, s, :],
                                     start=(s % 2 == 0), stop=(s % 2 == 1),
                                     skip_group_check=True)

                # finalize both chunks
                srec = finp.tile([P, 2, 4], FP, tag="srec", name=f"srec_{p}")
                nc.vector.tensor_scalar_add(srec[:], acc[:, :, 96:100], 1e-6)
                rec = finp.tile([P, 2, 4], FP, tag="rec", name=f"rec_{p}")
                nc.vector.reciprocal(rec[:], srec[:])
                os = finp.tile([P, 2, 24, 4], FP, tag="os", name=f"os_{p}")
                nc.vector.tensor_tensor(
                    out=os[:].rearrange("p c j h -> p c h j"),
                    in0=acc[:, :, 0:96].rearrange("p c (h j) -> p c h j", h=4),
                    in1=rec[:].unsqueeze(3).to_broadcast([P, 2, 4, 24]),
                    op=mybir.AluOpType.mult)
                o2 = finp.tile([P, 48], FP, tag="o2", name=f"o2_{p}")
                nc.vector.tensor_reduce(
                    out=o2[:].rearrange("p (c j) -> p c j", c=2),
                    in_=os[:], axis=mybir.AxisListType.X,
                    op=mybir.AluOpType.add)
                for ci in range(2):
                    k = 2 * p + ci
                    nc.sync.dma_start(out=out_d[k * P:(k + 1) * P, :],
                                      in_=o2[:, ci * 24:(ci + 1) * 24])

            for p in range(NP):
                front(p)
                if p >= 1:
                    tail(p - 1)
            tail(NP - 1)

    nc.compile()
    return nc


# ----------------------------------------------------------------------------
# public entry
# ----------------------------------------------------------------------------

_CACHE = {}
LAST_RUN = {}


def kernel(**inputs):
    x = np.asarray(inputs["x"])
    edge_attr = np.asarray(inputs["edge_attr"])
    edge_index = np.asarray(inputs["edge_index"])
    fw = _fold_weights(
        *[np.asarray(inputs[k], np.float32) for k in
          ("W_enc", "b_enc", "bn_gamma", "bn_beta", "bn_mean", "bn_var",
           "Wl", "bl", "Wr", "br", "We", "att", "bias_conv", "Wp", "bp")])

    in_maps, meta = _prepare(x, edge_attr, edge_index)
    NP = meta["NP"]

    if NP not in _CACHE:
        _CACHE[NP] = _build(NP)
    nc = _CACHE[NP]

    wmap = {
        "wencA": fw["wencA"].astype(np.float32),
        "wl2": fw["wl2"].astype(np.float32),
        "wr2": fw["wr2"].astype(np.float32),
        "we2": fw["we2"].astype(np.float32),
        "sgm": fw["sgm"].astype(np.float32),
        "wla": fw["wla"].astype(np.float32),
        "wra": fw["wra"].astype(np.float32),
        "wea": fw["wea"].astype(np.float32),
        "wfold": fw["wfold"].astype(np.float32),
        "sel4": fw["sel4"].astype(np.float32),
    }
    for im in in_maps:
        im.update(wmap)

    LAST_RUN["in_maps"] = in_maps
    LAST_RUN["nc"] = nc
    res = run_bass_kernel_spmd(nc, in_maps, core_ids=list(range(N_CORES)))

    # unshard
    N = x.shape[0]
    OUT = fw["OUT"]
    out = np.zeros((N, OUT), dtype=np.float32)
    for c, cd in enumerate(meta["cores"]):
        dev = np.asarray(res.results[c]["out"], np.float32)   # [NP*256, 24]
        for k, (nodes, e0, e1) in enumerate(cd["chunks"]):
            out[np.asarray(nodes)] = dev[k * P:k * P + len(nodes)]

    deg = np.bincount(np.asarray(edge_index[1], np.int64), minlength=N)
    sgn = (deg > 0).astype(np.float32)[:, None]
    out = out + sgn * fw["cbl"][None, :] + fw["cc"][None, :]
    return out.astype(np.float32)
